# revision 20
# baseline (speedup 1.0000x reference)
"""DeepSeek-style MLA attention block on 8 Trainium2 NeuronCores.

Sharding: tensor-parallel over the 16 heads (2 heads per core) through
attention; two head-sliced AllToAlls re-shard to token-parallel (512 tokens
per core) for the output projection; host concatenates the 8 token blocks.

Math notes (exact, no approximation beyond bf16 rounding):
  - g_q / g_kv layernorm gains are folded into the rows of w_qa / w_kva on
    the host.
  - LayerNorm is folded into the projections: the device projects RAW x^T
    (host-transposed, bf16) and corrects each projection z with
      z_corr = rstd ⊙ (z + (-mu) * colsum(W))
    where the -mu term rides as one extra K=1 matmul row and rstd is applied
    on the PSUM->SBUF copy (a copy that was needed anyway).  Token stats
    (mu, 1/std) come from bn_stats on a token-major bf16 copy of x.
  - 1/sqrt(head_dim) is folded into the Exp activation's scale.
  - Softmax skips the max-subtraction: logits are O(0.1) for this module's
    weight scale, so exp() is far from overflow.
  - Per-(head,token) softmax normalization is applied to y^T before the
    AllToAll via a gpsimd partition-broadcast of 1/sum.

All matmuls run in bf16 (1 cycle/row on the PE at any moving size); PSUM
accumulation stays fp32.  V stays SBUF-resident (no DRAM spill).  The
AllToAll is split per head so the second head's attention hides the first
collective, and the o_proj contraction is split into the matching halves so
its first half hides the second collective.
"""

import numpy as np

E = 2048
H = 16
D = 128
QR = 64
KR = 128
RD = 64
EPS = 1e-5
ROPE_BASE = 10000.0
B = 2
S = 2048
T = B * S            # 4096 total token rows
NCORES = 8
HPC = H // NCORES    # heads per core = 2
TBLK = T // NCORES   # tokens per core after re-shard = 512
NCH = 8              # token chunks of 512 in phase 1
CHK = 512
EXP_SCALE = float(1.0 / np.sqrt(D))

_CACHE = {}


def _build_module():
    import os
    import concourse.mybir as mybir
    import concourse.tile as tile
    from concourse import bacc
    from concourse.masks import make_identity

    F32 = mybir.dt.float32
    BF = mybir.dt.bfloat16
    AF = mybir.ActivationFunctionType
    ALU = mybir.AluOpType

    nc = bacc.Bacc("TRN2", target_bir_lowering=False, debug=False,
                   num_devices=NCORES)

    xtm_d = nc.declare_dram_parameter("xtm", [T, E], BF, isOutput=False)
    xfm_d = nc.declare_dram_parameter("xfm", [E, T], BF, isOutput=False)
    wqa_d = nc.declare_dram_parameter("wqa", [E, QR], BF, isOutput=False)
    wqb_d = nc.declare_dram_parameter("wqb", [QR, HPC * D], BF, isOutput=False)
    wkc_d = nc.declare_dram_parameter("wkc", [E, KR], BF, isOutput=False)
    wv_d = nc.declare_dram_parameter("wv", [E, HPC * D], BF, isOutput=False)
    wkvb_d = nc.declare_dram_parameter("wkvb", [KR, HPC * D], BF, isOutput=False)
    sw_d = nc.declare_dram_parameter("sw", [1, 448], BF, isOutput=False)
    wo_d = nc.declare_dram_parameter("wo", [E, E], BF, isOutput=False)
    cos_d = nc.declare_dram_parameter("cos_t", [RD, S], F32, isOutput=False)
    sin_d = nc.declare_dram_parameter("sin_t", [RD, S], F32, isOutput=False)
    mask_d = nc.declare_dram_parameter("masks", [128, 4, CHK], BF, isOutput=False)
    out_d = nc.declare_dram_parameter("out", [TBLK, E], F32, isOutput=True)

    with tile.TileContext(nc) as tc:
        with (
            tc.tile_pool(name="cons", bufs=1) as cons,
            tc.tile_pool(name="dram", bufs=1, space="DRAM") as dram,
        ):
            ident = cons.tile([128, 128], F32)
            make_identity(nc, ident)
            ident_b = cons.tile([128, 128], BF)
            nc.vector.tensor_copy(out=ident_b, in_=ident)
            ones_col = cons.tile([128, 1], BF)
            nc.vector.memset(ones_col, 1.0)
            eps_t = cons.tile([128, 1], F32)
            nc.vector.memset(eps_t, EPS)
            cos_f = cons.tile([RD, S], F32)
            nc.sync.dma_start(out=cos_f, in_=cos_d.ap())
            sin_e = cons.tile([RD, S], F32)
            nc.sync.dma_start(out=sin_e, in_=sin_d.ap())
            masks = cons.tile([128, 4, CHK], BF)
            nc.sync.dma_start(out=masks, in_=mask_d.ap())

            wqa = cons.tile([128, E // 128, QR], BF)
            nc.sync.dma_start(out=wqa, in_=wqa_d.ap().rearrange("(k p) m -> p k m", p=128))
            wkc = cons.tile([128, E // 128, KR], BF)
            nc.sync.dma_start(out=wkc, in_=wkc_d.ap().rearrange("(k p) m -> p k m", p=128))
            wv = cons.tile([128, E // 128, HPC * D], BF)
            nc.sync.dma_start(out=wv, in_=wv_d.ap().rearrange("(k p) m -> p k m", p=128))
            wqb = cons.tile([QR, HPC * D], BF)
            nc.sync.dma_start(out=wqb, in_=wqb_d.ap())
            wkvb = cons.tile([KR, HPC * D], BF)
            nc.sync.dma_start(out=wkvb, in_=wkvb_d.ap())
            sw = cons.tile([1, 448], BF)
            nc.sync.dma_start(out=sw, in_=sw_d.ap())

            # head-resident activations: feature-major Q^T/K^T, token-major V
            qT = [cons.tile([128, T], BF, name=f"qT{h}") for h in range(HPC)]
            kT = [cons.tile([128, T], BF, name=f"kT{h}") for h in range(HPC)]
            vtm = cons.tile([128, T // 128, HPC * D], BF, name="vtm")

            a2a_in = [dram.tile([NCORES, D, TBLK], BF, name=f"a2a_in{h}")
                      for h in range(HPC)]
            a2a_out = [dram.tile([NCORES, D, TBLK], BF, name=f"a2a_out{h}")
                       for h in range(HPC)]

            # ---------------- phase 1: stats + folded-LN projections ---------
            with (
                tc.tile_pool(name="p1sb", bufs=2) as p1,
                tc.tile_pool(name="p1ps", bufs=1, space="PSUM") as ps1,
            ):
                for c in (0, 1, 4, 5, 2, 3, 6, 7):
                    pos = (c % 4) * CHK       # position within the sequence
                    cs = slice(c * CHK, (c + 1) * CHK)

                    # x^T chunk (host-transposed); 4 DMAs to spread queues
                    xc = p1.tile([128, E // 128, CHK], BF, tag="xc", bufs=2)
                    for i in range(4):
                        nc.sync.dma_start(
                            out=xc[:, i * 4:(i + 1) * 4, :],
                            in_=xfm_d.ap()[i * 512:(i + 1) * 512, cs]
                                .rearrange("(k p) t -> p k t", p=128))

                    # per-token stats run under the stage-1 matmuls; half the
                    # tiles use DVE bn_stats, half the Act engine's fused
                    # accumulate (sum via Copy, sum-of-squares via Square) to
                    # balance the two engines
                    packs = []
                    for xt in range(4):
                        row0 = c * CHK + xt * 128
                        x_t = p1.tile([128, E], BF, tag="x", bufs=3)
                        nc.sync.dma_start(out=x_t, in_=xtm_d.ap()[row0:row0 + 128, :])
                        # pack2: col0 = rstd (-> broadcast source), col1 = -mu
                        # (-> matmul rhs row); bf16 so the transposed rows pack
                        # into one PSUM bank at free offsets 0 / CHK
                        pack2 = p1.tile([128, 2], BF, tag=f"pack{xt}")
                        std = p1.tile([128, 1], F32, tag="std")
                        if xt % 2 == 0:
                            stats = p1.tile([128, 4, 6], F32, tag="bst")
                            for g in range(4):
                                nc.vector.bn_stats(out=stats[:, g, :],
                                                   in_=x_t[:, g * 512:(g + 1) * 512])
                            mv = p1.tile([128, 2], F32, tag="mv")
                            nc.vector.bn_aggr(out=mv, in_=stats)
                            nc.vector.tensor_scalar(out=pack2[:, 1:2], in0=mv[:, 0:1],
                                                    scalar1=-1.0, scalar2=None,
                                                    op0=ALU.mult)
                            nc.scalar.activation(out=std, in_=mv[:, 1:2],
                                                 func=AF.Sqrt, bias=eps_t[:])
                        else:
                            scr = p1.tile([128, E], BF, tag="scr", bufs=2)
                            sum_t = p1.tile([128, 1], F32, tag="sum")
                            nc.scalar.activation(out=scr, in_=x_t, func=AF.Copy,
                                                 accum_out=sum_t[:])
                            sq_t = p1.tile([128, 1], F32, tag="sq")
                            nc.scalar.activation(out=scr, in_=x_t, func=AF.Square,
                                                 accum_out=sq_t[:])
                            nc.vector.tensor_scalar(out=pack2[:, 1:2], in0=sum_t,
                                                    scalar1=-1.0 / E, scalar2=None,
                                                    op0=ALU.mult)
                            mu2 = p1.tile([128, 1], F32, tag="mu2")
                            nc.vector.tensor_tensor(out=mu2, in0=pack2[:, 1:2],
                                                    in1=pack2[:, 1:2], op=ALU.mult)
                            var_t = p1.tile([128, 1], F32, tag="var")
                            nc.vector.scalar_tensor_tensor(
                                out=var_t, in0=sq_t, scalar=1.0 / E, in1=mu2,
                                op0=ALU.mult, op1=ALU.subtract)
                            nc.scalar.activation(out=std, in_=var_t,
                                                 func=AF.Sqrt, bias=eps_t[:])
                        rstd_f = p1.tile([128, 1], F32, tag="rstdf")
                        nc.vector.reciprocal(out=rstd_f, in_=std)
                        nc.vector.tensor_copy(out=pack2[:, 0:1], in_=rstd_f)
                        packs.append(pack2)

                    # stage 1: raw projections (independent of the stats)
                    zq = ps1.tile([QR, CHK], F32, tag="zq", bufs=1)
                    zk = ps1.tile([128, CHK], F32, tag="zk", bufs=1)
                    zv0 = ps1.tile([128, CHK], F32, tag="zv0", bufs=1)
                    zv1 = ps1.tile([128, CHK], F32, tag="zv1", bufs=1)
                    for k in range(E // 128):
                        rhs = xc[:, k, :]
                        st = (k == 0)
                        nc.tensor.matmul(zq[:], wqa[:, k, :], rhs, start=st, stop=False)
                        nc.tensor.matmul(zk[:], wkc[:, k, :], rhs, start=st, stop=False)
                        nc.tensor.matmul(zv0[:], wv[:, k, 0:128], rhs, start=st, stop=False)
                        nc.tensor.matmul(zv1[:], wv[:, k, 128:256], rhs, start=st, stop=False)

                    # transpose stats to a single PSUM row (rstd at cols 0:CHK,
                    # -mu at cols CHK:2*CHK), then the K=1 (-mu)*colsum rows
                    statsT_ps = ps1.tile([1, 2 * CHK], BF, tag="stats", bufs=1)
                    for xt in range(4):
                        nc.tensor.transpose(
                            statsT_ps[0:1, xt * 128:(xt + 1) * 128],
                            packs[xt][:, 0:1], ident_b[:])
                        nc.tensor.transpose(
                            statsT_ps[0:1, CHK + xt * 128:CHK + (xt + 1) * 128],
                            packs[xt][:, 1:2], ident_b[:])
                    rows2 = p1.tile([1, 2 * CHK], BF, tag="rows2", bufs=2)
                    nc.vector.tensor_copy(out=rows2, in_=statsT_ps)
                    negmu = rows2[0:1, CHK:2 * CHK]
                    rstd_bc = p1.tile([128, CHK], BF, tag="rstdbc", bufs=2)
                    nc.gpsimd.partition_broadcast(rstd_bc[:], rows2[0:1, 0:CHK])
                    nc.tensor.matmul(zq[:], sw[:, 0:64], negmu, start=False, stop=True)
                    nc.tensor.matmul(zk[:], sw[:, 64:192], negmu, start=False, stop=True)
                    nc.tensor.matmul(zv0[:], sw[:, 192:320], negmu, start=False, stop=True)
                    nc.tensor.matmul(zv1[:], sw[:, 320:448], negmu, start=False, stop=True)

                    # rstd on the PSUM->SBUF copies
                    qlow_sb = p1.tile([QR, CHK], BF, tag="qlow", bufs=1)
                    nc.vector.tensor_tensor(out=qlow_sb, in0=zq, in1=rstd_bc[0:QR, :],
                                            op=ALU.mult)
                    kc_sb = p1.tile([128, CHK], BF, tag="kc", bufs=1)
                    nc.vector.tensor_tensor(out=kc_sb, in0=zk, in1=rstd_bc, op=ALU.mult)
                    v_sbs = []
                    for h, zv in ((0, zv0), (1, zv1)):
                        v_sb = p1.tile([128, CHK], BF, tag=f"vsb{h}", bufs=1)
                        nc.vector.tensor_tensor(out=v_sb, in0=zv, in1=rstd_bc,
                                                op=ALU.mult)
                        v_sbs.append(v_sb)

                    # stage 2 + V transpose + RoPE per head
                    ps_slc = slice(pos, pos + CHK)
                    for h in range(HPC):
                        q_ps = ps1.tile([128, CHK], F32, tag="q2", bufs=1)
                        nc.tensor.matmul(q_ps[:], wqb[:, h * 128:(h + 1) * 128],
                                         qlow_sb[:], start=True, stop=True)
                        k_ps = ps1.tile([128, CHK], F32, tag="k2", bufs=1)
                        nc.tensor.matmul(k_ps[:], wkvb[:, h * 128:(h + 1) * 128],
                                         kc_sb[:], start=True, stop=True)
                        vtp4 = ps1.tile([128, 4, 128], BF, tag="vtp", bufs=1)
                        for i in range(4):
                            nc.tensor.transpose(vtp4[:, i, :],
                                                v_sbs[h][:, i * 128:(i + 1) * 128],
                                                ident_b[:])
                        if h == 0:
                            nc.scalar.copy(
                                out=vtm[:, c * 4:(c + 1) * 4, h * 128:(h + 1) * 128],
                                in_=vtp4)
                        else:
                            nc.vector.tensor_copy(
                                out=vtm[:, c * 4:(c + 1) * 4, h * 128:(h + 1) * 128],
                                in_=vtp4)

                        # RoPE: rotate via the PSUM->SBUF copy (partition swap
                        # on the Act engine), one gpsimd multiply, DVE mul+add
                        for src_ps, dstT in ((q_ps, qT[h]), (k_ps, kT[h])):
                            rot = p1.tile([RD, CHK], F32, tag="rot", bufs=2)
                            nc.scalar.copy(out=rot[0:32, :], in_=src_ps[32:64, :])
                            nc.scalar.copy(out=rot[32:64, :], in_=src_ps[0:32, :])
                            m1 = p1.tile([RD, CHK], BF, tag="m1", bufs=2)
                            nc.vector.tensor_tensor(out=m1, in0=src_ps[0:RD, :],
                                                    in1=cos_f[:, ps_slc], op=ALU.mult)
                            t2 = p1.tile([RD, CHK], BF, tag="t2", bufs=2)
                            nc.gpsimd.tensor_tensor(out=t2, in0=rot,
                                                    in1=sin_e[:, ps_slc], op=ALU.mult)
                            nc.vector.tensor_tensor(out=dstT[0:RD, cs], in0=m1,
                                                    in1=t2, op=ALU.add)
                            nc.scalar.copy(out=dstT[RD:128, cs], in_=src_ps[RD:128, :])

            # ---------------- phase 2: causal attention, head-major ----------
            with (
                tc.tile_pool(name="p2sb", bufs=1) as p2,
                tc.tile_pool(name="p2ps", bufs=1, space="PSUM") as ps2,
            ):
                for h in range(HPC):
                    for b in range(B):
                        boff = b * S
                        for g in range(2):
                            grp = [2 * g, 2 * g + 1]
                            y_ps = {qc: ps2.tile([128, CHK], F32, tag=f"y{qc % 2}",
                                                 bufs=1, name=f"ps_y{qc}")
                                    for qc in grp}
                            sums_ps = {qc: ps2.tile([1, CHK], F32, tag=f"sums{qc % 2}",
                                                    bufs=1, name=f"ps_sums{qc}")
                                       for qc in grp}
                            kt_max = 4 * grp[-1] + 3
                            for kt in range(kt_max + 1):
                                kslc = slice(boff + kt * 128, boff + (kt + 1) * 128)
                                qcs = [qc for qc in grp if kt <= 4 * qc + 3]
                                att = {}
                                for qc in qcs:
                                    # diagonal blocks only need columns >= 128d
                                    # (everything to the left is masked out)
                                    d = kt - 4 * qc
                                    lo = 128 * d if 0 <= d <= 3 else 0
                                    q0 = boff + qc * CHK
                                    s_ps = ps2.tile([128, CHK], F32, tag="s", bufs=3)
                                    nc.tensor.matmul(s_ps[:, lo:], kT[h][:, kslc],
                                                     qT[h][:, q0 + lo:q0 + CHK],
                                                     start=True, stop=True)
                                    a_t = p2.tile([128, CHK], BF, tag=f"att{qc % 2}",
                                                  bufs=3, name=f"att{qc}")
                                    nc.scalar.activation(out=a_t[:, lo:],
                                                         in_=s_ps[:, lo:],
                                                         func=AF.Exp,
                                                         scale=EXP_SCALE)
                                    if 0 <= d <= 3:
                                        nc.vector.tensor_tensor(out=a_t[:, lo:],
                                                                in0=a_t[:, lo:],
                                                                in1=masks[:, d, lo:],
                                                                op=ALU.mult)
                                    att[qc] = (a_t, lo)
                                for qc in qcs:
                                    a_t, lo = att[qc]
                                    nc.tensor.matmul(
                                        sums_ps[qc][:, lo:], ones_col[:],
                                        a_t[:, lo:],
                                        start=(kt == 0), stop=(kt == 4 * qc + 3))
                                for qc in qcs:
                                    a_t, lo = att[qc]
                                    nc.tensor.matmul(
                                        y_ps[qc][:, lo:],
                                        vtm[:, b * 16 + kt, h * 128:(h + 1) * 128],
                                        a_t[:, lo:],
                                        start=(kt == 0), stop=(kt == 4 * qc + 3))

                            for qc in grp:
                                recip = p2.tile([1, CHK], F32, tag="recip", bufs=2)
                                nc.vector.reciprocal(out=recip, in_=sums_ps[qc])
                                recip_b = p2.tile([1, CHK], BF, tag="recipb", bufs=2)
                                nc.vector.tensor_copy(out=recip_b, in_=recip)
                                bc_sb = p2.tile([128, CHK], BF, tag="bc", bufs=2)
                                nc.gpsimd.partition_broadcast(bc_sb[:], recip_b[:])
                                ynorm = p2.tile([128, CHK], BF, tag="ynorm", bufs=2)
                                nc.vector.tensor_tensor(out=ynorm, in0=y_ps[qc],
                                                        in1=bc_sb, op=ALU.mult)
                                nc.sync.dma_start(
                                    out=a2a_in[h][b * 4 + qc, :, :], in_=ynorm)

                    # re-shard this head's output while the next head computes
                    if os.environ.get("KERNEL_NO_COLLECTIVE"):
                        nc.sync.dma_start(out=a2a_out[h][:], in_=a2a_in[h][:])
                    else:
                        nc.gpsimd.collective_compute(
                            "AllToAll", mybir.AluOpType.bypass,
                            replica_groups=[list(range(NCORES))],
                            ins=[a2a_in[h].opt()],
                            outs=[a2a_out[h].opt()],
                        )

            # ---------------- phase 4: output projection --------------------
            with (
                tc.tile_pool(name="p4sb", bufs=1) as p4,
                tc.tile_pool(name="p4w", bufs=3) as p4w,
                tc.tile_pool(name="p4ps", bufs=1, space="PSUM") as ps4,
            ):
                ya = [p4.tile([128, NCORES, TBLK], BF, name=f"ya{h}")
                      for h in range(HPC)]
                for h in range(HPC):
                    for i in range(2):
                        nc.sync.dma_start(
                            out=ya[h][:, i * 4:(i + 1) * 4, :],
                            in_=a2a_out[h][i * 4:(i + 1) * 4]
                                .rearrange("c p t -> p c t"))
                for half in range(2):
                    o_ps = [[ps4.tile([128, 512], F32, tag=f"o{mt}{nt}", bufs=1,
                                      name=f"ps_o{mt}{nt}")
                             for nt in range(2)] for mt in range(4)]
                    # contraction split by head so half the matmuls can start
                    # as soon as the first AllToAll has landed
                    for h in range(HPC):
                        for c in range(NCORES):
                            krow = (2 * c + h) * 128
                            wo_t = p4w.tile([128, 1024], BF, tag="wo")
                            nc.sync.dma_start(
                                out=wo_t,
                                in_=wo_d.ap()[krow:krow + 128,
                                              half * 1024:(half + 1) * 1024])
                            for mt in range(4):
                                for nt in range(2):
                                    nc.tensor.matmul(
                                        o_ps[mt][nt][:],
                                        ya[h][:, c, mt * 128:(mt + 1) * 128],
                                        wo_t[:, nt * 512:(nt + 1) * 512],
                                        start=(h == 0 and c == 0),
                                        stop=(h == 1 and c == NCORES - 1))
                    for mt in range(4):
                        for nt in range(2):
                            o_sb = p4.tile([128, 512], F32, tag="o_sb", bufs=4)
                            if (mt + nt) % 2 == 0:
                                nc.scalar.copy(out=o_sb, in_=o_ps[mt][nt])
                            else:
                                nc.vector.tensor_copy(out=o_sb, in_=o_ps[mt][nt])
                            nc.sync.dma_start(
                                out=out_d.ap()[mt * 128:(mt + 1) * 128,
                                               half * 1024 + nt * 512:
                                               half * 1024 + (nt + 1) * 512],
                                in_=o_sb)

    nc.compile()
    return nc


def _host_inputs(x, g_q, g_kv, w_qa, w_qb, w_kva, w_kvb, w_o):
    import ml_dtypes
    BF16 = ml_dtypes.bfloat16

    x_flat = np.asarray(x, dtype=np.float32).reshape(T, E)
    xtm = np.ascontiguousarray(x_flat.astype(BF16))
    xfm = np.ascontiguousarray(x_flat.T.astype(BF16))
    wqa_g = (np.asarray(w_qa, dtype=np.float32) * np.asarray(g_q, np.float32)[:, None])
    wkva_g = (np.asarray(w_kva, dtype=np.float32) * np.asarray(g_kv, np.float32)[:, None])
    wkc = wkva_g[:, :KR]
    wo = np.ascontiguousarray(np.asarray(w_o, np.float32).astype(BF16))

    inv_freq = 1.0 / (ROPE_BASE ** (np.arange(0, RD, 2, dtype=np.float32) / RD))
    freqs = np.arange(S, dtype=np.float32)[:, None] * inv_freq[None, :]  # [S, 32]
    cos_half = np.cos(freqs).T                                           # [32, S]
    sin_half = np.sin(freqs).T
    cos_t = np.ascontiguousarray(np.concatenate([cos_half, cos_half], axis=0))
    sin_t = np.ascontiguousarray(np.concatenate([-sin_half, sin_half], axis=0))

    ii = np.arange(128)[:, None, None]
    dd = np.arange(4)[None, :, None]
    jj = np.arange(CHK)[None, None, :]
    masks = ((ii + 128 * dd) <= jj).astype(BF16)

    in_maps = []
    for c in range(NCORES):
        h0 = HPC * c
        wqb_c = w_qb[:, h0 * D:(h0 + HPC) * D].astype(np.float32)
        wkvb_c = w_kvb[:, h0 * D:(h0 + HPC) * D].astype(np.float32)
        vcols = []
        for h in (h0, h0 + 1):
            vcols.append(wkva_g[:, KR + 2 * D * h: KR + 2 * D * h + D])
        wv_c = np.concatenate(vcols, axis=1)
        sw_c = np.concatenate([wqa_g.sum(axis=0), wkc.sum(axis=0),
                               wv_c.sum(axis=0)])[None, :]
        in_maps.append({
            "xtm": xtm, "xfm": xfm,
            "wqa": np.ascontiguousarray(wqa_g.astype(BF16)),
            "wqb": np.ascontiguousarray(wqb_c.astype(BF16)),
            "wkc": np.ascontiguousarray(wkc.astype(BF16)),
            "wv": np.ascontiguousarray(wv_c.astype(BF16)),
            "wkvb": np.ascontiguousarray(wkvb_c.astype(BF16)),
            "sw": np.ascontiguousarray(sw_c.astype(BF16)),
            "wo": wo, "cos_t": cos_t, "sin_t": sin_t, "masks": masks,
        })
    return in_maps


def kernel(x, g_q, g_kv, w_qa, w_qb, w_kva, w_kvb, w_o):
    from concourse.bass_utils import run_bass_kernel_spmd

    if "nc" not in _CACHE:
        _CACHE["nc"] = _build_module()
    nc = _CACHE["nc"]

    in_maps = _host_inputs(np.asarray(x), np.asarray(g_q), np.asarray(g_kv),
                           np.asarray(w_qa), np.asarray(w_qb),
                           np.asarray(w_kva), np.asarray(w_kvb),
                           np.asarray(w_o))
    res = run_bass_kernel_spmd(nc, in_maps, list(range(NCORES)))
    blocks = [res.results[c]["out"] for c in range(NCORES)]
    return np.concatenate(blocks, axis=0).reshape(B, S, E).astype(np.float32)
